# revision 14
# baseline (speedup 1.0000x reference)
"""Trainium2 Bass kernel for nn_GCN_31585189495371.

3-layer GCN over 256 independent 400-node graphs, per-graph flatten ->
linear -> logits.  Formulated with a dense per-graph weighted adjacency
S [src, dst] (built host-side from the COO edge list; pure layout
transform + duplicate-edge accumulation) so that message passing is a
dense matmul: z = S^T @ u.  Symmetric normalization D = diag(deg^-1/2)
is applied on-device via per-partition scales; biases enter the psum
accumulation as a rank-1 (sqrt(deg) x b) augmented-row matmul.

Per-core work: 32 graphs (graph-level data parallelism across the 8
NeuronCores, as in the sharding hint).  Graphs are processed in pairs,
with each graph of a pair occupying one 64-column group of the 128x128
PE array (tile_position col-tiling) so the 64-feature matmuls run two
graphs concurrently at full array width.

Identity used by kernel (derived from the reference):
  u0   = D (x W1)
  z_k  = S^T u_{k-1} (+ sqrt(deg) (x) b_k   for k=1,2 via aug row)
  a_k  = relu(z_k)              (true h_k = D a_k, D deferred)
  u_k  = D^2 (a_k W_{k+1})      (D^2 because h_k carries a deferred D)
  h3   = D z_3                  (b3 folded into the readout bias)
  out  = flatten(h3) @ Wc' ... @ Wl + bl'   (bc, b3 folded host-side)
"""

import sys

sys.path.insert(0, "/opt/trn_rl_repo")

from contextlib import ExitStack

import numpy as np
import ml_dtypes

from concourse import bacc, bass, mybir
import concourse.tile as tile
from concourse.bass_utils import run_bass_kernel_spmd

BF = ml_dtypes.bfloat16
G, NPG, FIN, H = 256, 400, 400, 64
NCORES = 8
GPC = G // NCORES          # graphs per core
# node-dim chunks of 128 (contraction tiling)
KCH = [(0, 128), (128, 128), (256, 128), (384, 16)]

_dt = mybir.dt


def _emit(nc: bass.Bass, gpc: int):
    """Emit the per-core Tile program. Same program runs SPMD on all cores."""
    pairs = gpc // 2

    xT = nc.dram_tensor("xT", [gpc, FIN, NPG], _dt.bfloat16, kind="ExternalInput").ap()
    Sa = nc.dram_tensor("Sa", [gpc, NPG + 1, NPG], _dt.bfloat16, kind="ExternalInput").ap()
    dck = nc.dram_tensor("dck", [4, 128, 2 * gpc], _dt.float32, kind="ExternalInput").ap()
    dr = nc.dram_tensor("dr", [pairs, 2, NPG], _dt.bfloat16, kind="ExternalInput").ap()
    w1 = nc.dram_tensor("w1", [FIN, H], _dt.bfloat16, kind="ExternalInput").ap()
    w23 = nc.dram_tensor("w23", [2, H, H], _dt.bfloat16, kind="ExternalInput").ap()
    bia = nc.dram_tensor("bia", [2, 128], _dt.bfloat16, kind="ExternalInput").ap()
    wcp = nc.dram_tensor("wcp", [128, 200 * H], _dt.bfloat16, kind="ExternalInput").ap()
    wl = nc.dram_tensor("wl", [H, 2], _dt.bfloat16, kind="ExternalInput").ap()
    blr = nc.dram_tensor("blr", [1, 2], _dt.bfloat16, kind="ExternalInput").ap()
    idn = nc.dram_tensor("idn", [128, 128], _dt.bfloat16, kind="ExternalInput").ap()
    c01 = nc.dram_tensor("c01", [2, 128], _dt.bfloat16, kind="ExternalInput").ap()
    one = nc.dram_tensor("one", [1, gpc], _dt.bfloat16, kind="ExternalInput").ap()
    out = nc.dram_tensor("out", [gpc, 2], _dt.float32, kind="ExternalOutput").ap()

    AF = mybir.ActivationFunctionType
    OP = mybir.AluOpType

    with tile.TileContext(nc) as tc, ExitStack() as ctx:
        const = ctx.enter_context(tc.tile_pool(name="const", bufs=1))
        inp = ctx.enter_context(tc.tile_pool(name="inp", bufs=2))
        act = ctx.enter_context(tc.tile_pool(name="act", bufs=2))
        un_p = ctx.enter_context(tc.tile_pool(name="un", bufs=2))
        psz = ctx.enter_context(tc.tile_pool(name="psz", bufs=3, space="PSUM"))
        pstr = ctx.enter_context(tc.tile_pool(name="pstr", bufs=2, space="PSUM"))

        # ---- constants (one-time loads) ----
        w1_t = []
        for i, (o, k) in enumerate(KCH):
            t = const.tile([k, H], _dt.bfloat16, name=f"w1c{i}")
            nc.sync.dma_start(t[:], w1[o : o + k, :])
            w1_t.append(t)
        w2_t = const.tile([128, H], _dt.bfloat16, name="w2c")
        nc.sync.dma_start(w2_t[0:64, :], w23[0])
        nc.sync.dma_start(w2_t[64:128, :], w23[0])
        w3_t = const.tile([128, H], _dt.bfloat16, name="w3c")
        nc.sync.dma_start(w3_t[0:64, :], w23[1])
        nc.sync.dma_start(w3_t[64:128, :], w23[1])
        dck_t = []
        for j in range(4):
            t = const.tile([128, 2 * gpc], _dt.float32, name=f"dckc{j}")
            nc.sync.dma_start(t[:], dck[j])
            dck_t.append(t)
        idn_t = const.tile([128, 128], _dt.bfloat16, name="idnc")
        nc.sync.dma_start(idn_t[:], idn[:])
        c01_t = const.tile([2, 128], _dt.bfloat16, name="c01c")
        nc.sync.dma_start(c01_t[:], c01[:])
        wl_t = const.tile([H, 2], _dt.bfloat16, name="wlc")
        nc.sync.dma_start(wl_t[:], wl[:])
        blr_t = const.tile([1, 2], _dt.bfloat16, name="blrc")
        nc.sync.dma_start(blr_t[:], blr[:])
        one_t = const.tile([1, gpc], _dt.bfloat16, name="onec")
        nc.sync.dma_start(one_t[:], one[:])
        wcp_t = const.tile([128, 200 * H], _dt.bfloat16, name="wcpc")
        nc.sync.dma_start(wcp_t[:], wcp[:])
        flat = const.tile([128, gpc * 200], _dt.bfloat16, name="flatc")

        SCH = [(0, 128), (128, 128), (256, 128), (384, 17)]  # S chunks incl aug row

        for p in range(pairs):
            ga, gb = 2 * p, 2 * p + 1
            xa, xb, sa, sb = [], [], [], []
            for i, (o, k) in enumerate(KCH):
                t = inp.tile([k, NPG], _dt.bfloat16, name=f"xa{i}", tag=f"xa{i}")
                nc.sync.dma_start(t[:], xT[ga, o : o + k, :])
                xa.append(t)
                t = inp.tile([k, NPG], _dt.bfloat16, name=f"xb{i}", tag=f"xb{i}")
                nc.sync.dma_start(t[:], xT[gb, o : o + k, :])
                xb.append(t)
            for i, (o, k) in enumerate(SCH):
                t = inp.tile([k, NPG], _dt.bfloat16, name=f"sa{i}", tag=f"sa{i}")
                nc.sync.dma_start(t[:], Sa[ga, o : o + k, :])
                sa.append(t)
                t = inp.tile([k, NPG], _dt.bfloat16, name=f"sb{i}", tag=f"sb{i}")
                nc.sync.dma_start(t[:], Sa[gb, o : o + k, :])
                sb.append(t)
            drp = inp.tile([2, NPG], _dt.bfloat16, name="drp", tag="drp")
            nc.sync.dma_start(drp[:], dr[p])

            # ---- L1: tT = (x W1)^T, pair-packed in column groups ----
            ps = psz.tile([128, NPG], _dt.float32, name="psl1", tag="z")
            for i, (o, k) in enumerate(KCH):
                nc.tensor.matmul(ps[0:64, :], w1_t[i][:], xa[i][:],
                                 start=(i == 0), stop=(i == 3), tile_position=(0, 0))
            for i, (o, k) in enumerate(KCH):
                nc.tensor.matmul(ps[64:128, :], w1_t[i][:], xb[i][:],
                                 start=(i == 0), stop=(i == 3), tile_position=(0, 64))

            for layer in (1, 2, 3):
                # evacuate tT (psum -> sbuf bf16); plain copy, D deferred
                tT = act.tile([128, NPG], _dt.bfloat16, name="tT", tag="tT")
                nc.scalar.activation(tT[:], ps[:], AF.Copy)

                # transpose chunks to node layout + evacuate with D scale
                pw = gpc if layer > 1 else 0  # dinv^2 cols for layers 2,3
                un = []
                for j, (o, k) in enumerate(KCH):
                    pt = pstr.tile([128, 128], _dt.bfloat16, name=f"pt{j}", tag=f"tr{j % 2}")
                    nc.tensor.transpose(pt[0:k, :], tT[:, o : o + k], idn_t[:])
                    rows = 17 if (j == 3 and layer < 3) else k
                    ut = un_p.tile([rows, 128], _dt.bfloat16, name=f"un{j}", tag=f"un{j}")
                    nc.vector.tensor_scalar(
                        ut[0:k, 0:64], pt[0:k, 0:64],
                        dck_t[j][0:k, pw + ga : pw + ga + 1], None, OP.mult)
                    nc.vector.tensor_scalar(
                        ut[0:k, 64:128], pt[0:k, 64:128],
                        dck_t[j][0:k, pw + gb : pw + gb + 1], None, OP.mult)
                    un.append(ut)
                if layer < 3:  # bias goes into the aug row of the tail chunk
                    nc.sync.dma_start(un[3][16:17, :], bia[layer - 1 : layer, :])

                # S-message matmul: z^T = u^T S (pair in column groups)
                ps2 = psz.tile([128, NPG], _dt.float32, name="psz2", tag="z")
                ntail = 17 if layer < 3 else 16
                for j in range(4):
                    k = KCH[j][1] if j < 3 else ntail
                    nc.tensor.matmul(ps2[0:64, :], un[j][0:k, 0:64], sa[j][0:k, :],
                                     start=(j == 0), stop=(j == 3), tile_position=(0, 0))
                for j in range(4):
                    k = KCH[j][1] if j < 3 else ntail
                    nc.tensor.matmul(ps2[64:128, :], un[j][0:k, 64:128], sb[j][0:k, :],
                                     start=(j == 0), stop=(j == 3), tile_position=(0, 64))

                if layer < 3:
                    aT = act.tile([128, NPG], _dt.bfloat16, name="aT", tag="aT")
                    nc.scalar.activation(aT[:], ps2[:], AF.Relu)
                    wt = w2_t if layer == 1 else w3_t
                    ps = psz.tile([128, NPG], _dt.float32, name="psw", tag="z")
                    nc.tensor.matmul(ps[0:64, :], wt[0:64, :], aT[0:64, :],
                                     start=True, stop=True, tile_position=(0, 0))
                    nc.tensor.matmul(ps[64:128, :], wt[64:128, :], aT[64:128, :],
                                     start=True, stop=True, tile_position=(64, 64))
                else:
                    # h3^T = z3^T * dinv (broadcast over feature partitions)
                    pd = pstr.tile([128, NPG], _dt.float32, name="pd", tag="pd", bufs=1)
                    nc.tensor.matmul(pd[:], c01_t[:], drp[:], start=True, stop=True)
                    dvb = act.tile([128, NPG], _dt.float32, name="dvb", tag="dvb")
                    nc.scalar.activation(dvb[:], pd[:], AF.Copy)
                    h3 = act.tile([128, NPG], _dt.bfloat16, name="h3", tag="h3")
                    nc.vector.tensor_tensor(h3[:], ps2[:], dvb[:], OP.mult)
                    # flatten: contiguous block copies into the readout tile
                    nc.sync.dma_start(flat[0:64, ga * 200 : ga * 200 + 200], h3[0:64, 0:200])
                    nc.sync.dma_start(flat[64:128, ga * 200 : ga * 200 + 200], h3[0:64, 200:400])
                    nc.sync.dma_start(flat[0:64, gb * 200 : gb * 200 + 200], h3[64:128, 0:200])
                    nc.sync.dma_start(flat[64:128, gb * 200 : gb * 200 + 200], h3[64:128, 200:400])

        # ---- readout: g = flat' . Wc' (contract 25600 in 200 chunks) ----
        flat_r = flat[:].rearrange("p (g c) -> p c g", g=gpc)
        gps = pstr.tile([gpc, H], _dt.float32, name="gps", tag="pd", bufs=1)
        for c in range(200):
            nc.tensor.matmul(gps[:], flat_r[:, c, :], wcp_t[:, c * H : (c + 1) * H],
                             start=(c == 0), stop=(c == 199))
        gsb = const.tile([gpc, H], _dt.bfloat16, name="gsb")
        nc.scalar.activation(gsb[:], gps[:], AF.Copy)
        gtp = pstr.tile([H, gpc], _dt.bfloat16, name="gtp", tag="tr0")
        nc.tensor.transpose(gtp[:], gsb[:], idn_t[0:gpc, 0:gpc])
        gts = const.tile([H, gpc], _dt.bfloat16, name="gts")
        nc.scalar.activation(gts[:], gtp[:], AF.Copy)
        ops = pstr.tile([gpc, 2], _dt.float32, name="ops", tag="tr1")
        nc.tensor.matmul(ops[:], gts[:], wl_t[:], start=True, stop=False)
        nc.tensor.matmul(ops[:], one_t[:], blr_t[:], start=False, stop=True)
        osb = const.tile([gpc, 2], _dt.float32, name="osb")
        nc.scalar.activation(osb[:], ops[:], AF.Copy)
        nc.sync.dma_start(out[:], osb[:])

    return nc


def build(gpc: int = GPC) -> bass.Bass:
    nc = bacc.Bacc("TRN2", target_bir_lowering=False, debug=False)
    _emit(nc, gpc)
    nc.compile()
    return nc


def prep_inputs(x, edge_index, edge_weight, W1, b1, W2, b2, W3, b3, Wc, bc, Wl, bl,
                gpc: int = GPC, ncores: int = NCORES):
    """Host-side prep: dense adjacency, normalization constants, layout."""
    f32 = np.float32
    x = np.asarray(x, f32)
    edge_index = np.asarray(edge_index)
    edge_weight = np.asarray(edge_weight, f32)
    W1, b1 = np.asarray(W1, f32), np.asarray(b1, f32)
    W2, b2 = np.asarray(W2, f32), np.asarray(b2, f32)
    W3, b3 = np.asarray(W3, f32), np.asarray(b3, f32)
    Wc, bc = np.asarray(Wc, f32), np.asarray(bc, f32)
    Wl, bl = np.asarray(Wl, f32), np.asarray(bl, f32)

    ng = gpc * ncores
    n = ng * NPG
    src, dst = edge_index[0], edge_index[1]
    S = np.zeros((n, NPG), f32)
    np.add.at(S, (src, dst - (src // NPG) * NPG), edge_weight)
    S[np.arange(n), np.arange(n) % NPG] += 1.0
    S3 = S.reshape(ng, NPG, NPG)
    deg = S3.sum(axis=1)
    dinv = (1.0 / np.sqrt(deg)).astype(f32)
    sqd = np.sqrt(deg).astype(f32)

    Sa = np.concatenate([S3, sqd[:, None, :]], axis=1).astype(BF)  # [ng,401,400]
    xT = np.ascontiguousarray(
        x.reshape(ng, NPG, FIN).transpose(0, 2, 1)).astype(BF)     # [ng,400,400]

    # dck [4, 128, 2*gpc] per core: dinv cols then dinv^2 cols
    pairs = gpc // 2
    dck_full = np.zeros((ncores, 4, 128, 2 * gpc), f32)
    dr_full = np.zeros((ncores, pairs, 2, NPG), f32)
    for c in range(ncores):
        dv = dinv[c * gpc : (c + 1) * gpc]          # [gpc, 400]
        for j, (o, k) in enumerate(KCH):
            dck_full[c, j, 0:k, 0:gpc] = dv[:, o : o + k].T
            dck_full[c, j, 0:k, gpc : 2 * gpc] = (dv * dv)[:, o : o + k].T
        dr_full[c] = dv.reshape(pairs, 2, NPG)

    # folded biases
    bc_p = bc + (np.tile(b3, NPG) @ Wc)
    bl_p = (bl + bc_p @ Wl).reshape(1, 2)

    # Wc' reorder to match device flat layout: chunk c rows 0:64 = node c,
    # rows 64:128 = node 200+c (features in order)
    Wcr = Wc.reshape(NPG, H, H)
    Wcp = np.zeros((200, 128, H), f32)
    Wcp[:, 0:64, :] = Wcr[0:200]
    Wcp[:, 64:128, :] = Wcr[200:400]
    wcp = np.ascontiguousarray(Wcp.transpose(1, 0, 2)).reshape(128, 200 * H).astype(BF)

    bia = np.zeros((2, 128), f32)
    bia[0] = np.concatenate([b1, b1])
    bia[1] = np.concatenate([b2, b2])

    c01 = np.zeros((2, 128), f32)
    c01[0, 0:64] = 1.0
    c01[1, 64:128] = 1.0

    consts = dict(
        w1=W1.astype(BF),
        w23=np.stack([W2, W3]).astype(BF),
        bia=bia.astype(BF),
        wcp=wcp,
        wl=Wl.astype(BF),
        blr=bl_p.astype(BF),
        idn=np.eye(128, dtype=f32).astype(BF),
        c01=c01.astype(BF),
        one=np.ones((1, gpc), f32).astype(BF),
    )

    in_maps = []
    for c in range(ncores):
        m = dict(consts)
        m["xT"] = xT[c * gpc : (c + 1) * gpc]
        m["Sa"] = Sa[c * gpc : (c + 1) * gpc]
        m["dck"] = dck_full[c]
        m["dr"] = dr_full[c].astype(BF)
        in_maps.append(m)
    return in_maps


_NC_CACHE = {}


def kernel(x, edge_index, edge_weight, W1, b1, W2, b2, W3, b3, Wc, bc, Wl, bl,
           _trace=False, _trace_kwargs=None):
    in_maps = prep_inputs(x, edge_index, edge_weight, W1, b1, W2, b2, W3, b3,
                          Wc, bc, Wl, bl)
    if GPC not in _NC_CACHE:
        _NC_CACHE[GPC] = build(GPC)
    nc = _NC_CACHE[GPC]
    res = run_bass_kernel_spmd(
        nc, in_maps, core_ids=list(range(NCORES)),
        trace=_trace, **(_trace_kwargs or {}))
    outs = np.concatenate([r["out"] for r in res.results], axis=0)
    if _trace:
        return outs.astype(np.float32), res
    return outs.astype(np.float32)


# revision 17
# speedup vs baseline: 1.2096x; 1.2096x over previous
"""Trainium2 Bass kernel for nn_GCN_31585189495371.

3-layer GCN over 256 independent 400-node graphs, per-graph flatten ->
linear -> logits.  Formulated with a dense per-graph weighted adjacency
S [src, dst] (built host-side from the COO edge list; pure layout
transform + duplicate-edge accumulation) so that message passing is a
dense matmul: z = S^T @ u.  Symmetric normalization D = diag(deg^-1/2)
is applied on-device via per-partition scales; biases enter the psum
accumulation as a rank-1 (sqrt(deg) x b) augmented-row matmul.

Per-core work: 32 graphs (graph-level data parallelism across the 8
NeuronCores, as in the sharding hint).  Graphs are processed in pairs,
with each graph of a pair occupying one 64-column group of the 128x128
PE array (tile_position col-tiling) so the 64-feature matmuls run two
graphs concurrently at full array width.

Identity used by kernel (derived from the reference):
  u0   = D (x W1)
  z_k  = S^T u_{k-1} (+ sqrt(deg) (x) b_k   for k=1,2 via aug row)
  a_k  = relu(z_k)              (true h_k = D a_k, D deferred)
  u_k  = D^2 (a_k W_{k+1})      (D^2 because h_k carries a deferred D)
  h3   = D z_3                  (b3 folded into the readout bias)
  out  = flatten(h3) @ Wc' ... @ Wl + bl'   (bc, b3 folded host-side)
"""

import sys

sys.path.insert(0, "/opt/trn_rl_repo")

from contextlib import ExitStack

import numpy as np
import ml_dtypes

from concourse import bacc, bass, mybir
import concourse.tile as tile
from concourse.bass_utils import run_bass_kernel_spmd

BF = ml_dtypes.bfloat16
G, NPG, FIN, H = 256, 400, 400, 64
NCORES = 8
GPC = G // NCORES          # graphs per core
# node-dim chunks of 128 (contraction tiling)
KCH = [(0, 128), (128, 128), (256, 128), (384, 16)]

_dt = mybir.dt


def _emit(nc: bass.Bass, gpc: int):
    """Emit the per-core Tile program. Same program runs SPMD on all cores."""
    pairs = gpc // 2

    xT = nc.dram_tensor("xT", [gpc, FIN, NPG], _dt.bfloat16, kind="ExternalInput").ap()
    Sa = nc.dram_tensor("Sa", [gpc, NPG + 1, NPG], _dt.bfloat16, kind="ExternalInput").ap()
    dck = nc.dram_tensor("dck", [4, 128, 2 * gpc], _dt.float32, kind="ExternalInput").ap()
    dr = nc.dram_tensor("dr", [pairs, 2, NPG], _dt.bfloat16, kind="ExternalInput").ap()
    w1 = nc.dram_tensor("w1", [FIN, H], _dt.bfloat16, kind="ExternalInput").ap()
    w23 = nc.dram_tensor("w23", [2, H, H], _dt.bfloat16, kind="ExternalInput").ap()
    bia = nc.dram_tensor("bia", [2, 128], _dt.bfloat16, kind="ExternalInput").ap()
    wcp = nc.dram_tensor("wcp", [128, 200 * H], _dt.bfloat16, kind="ExternalInput").ap()
    wl = nc.dram_tensor("wl", [H, 2], _dt.bfloat16, kind="ExternalInput").ap()
    blr = nc.dram_tensor("blr", [1, 2], _dt.bfloat16, kind="ExternalInput").ap()
    idn = nc.dram_tensor("idn", [128, 128], _dt.bfloat16, kind="ExternalInput").ap()
    c01 = nc.dram_tensor("c01", [2, 128], _dt.bfloat16, kind="ExternalInput").ap()
    one = nc.dram_tensor("one", [1, gpc], _dt.bfloat16, kind="ExternalInput").ap()
    out = nc.dram_tensor("out", [gpc, 2], _dt.float32, kind="ExternalOutput").ap()

    AF = mybir.ActivationFunctionType
    OP = mybir.AluOpType

    with tile.TileContext(nc) as tc, ExitStack() as ctx:
        const = ctx.enter_context(tc.tile_pool(name="const", bufs=1))
        inp = ctx.enter_context(tc.tile_pool(name="inp", bufs=3))
        act = ctx.enter_context(tc.tile_pool(name="act", bufs=3))
        un_p = ctx.enter_context(tc.tile_pool(name="un", bufs=3))
        psz = ctx.enter_context(tc.tile_pool(name="psz", bufs=3, space="PSUM"))
        pstr = ctx.enter_context(tc.tile_pool(name="pstr", bufs=2, space="PSUM"))

        # ---- constants (one-time loads) ----
        w1_t = []
        for i, (o, k) in enumerate(KCH):
            t = const.tile([k, H], _dt.bfloat16, name=f"w1c{i}")
            nc.sync.dma_start(t[:], w1[o : o + k, :])
            w1_t.append(t)
        w2_t = const.tile([128, H], _dt.bfloat16, name="w2c")
        nc.sync.dma_start(w2_t[0:64, :], w23[0])
        nc.sync.dma_start(w2_t[64:128, :], w23[0])
        w3_t = const.tile([128, H], _dt.bfloat16, name="w3c")
        nc.sync.dma_start(w3_t[0:64, :], w23[1])
        nc.sync.dma_start(w3_t[64:128, :], w23[1])
        dck_t = []
        for j in range(4):
            t = const.tile([128, 2 * gpc], _dt.float32, name=f"dckc{j}")
            nc.sync.dma_start(t[:], dck[j])
            dck_t.append(t)
        idn_t = const.tile([128, 128], _dt.bfloat16, name="idnc")
        nc.sync.dma_start(idn_t[:], idn[:])
        c01_t = const.tile([2, 128], _dt.bfloat16, name="c01c")
        nc.sync.dma_start(c01_t[:], c01[:])
        wl_t = const.tile([H, 2], _dt.bfloat16, name="wlc")
        nc.sync.dma_start(wl_t[:], wl[:])
        blr_t = const.tile([1, 2], _dt.bfloat16, name="blrc")
        nc.sync.dma_start(blr_t[:], blr[:])
        one_t = const.tile([1, gpc], _dt.bfloat16, name="onec")
        nc.sync.dma_start(one_t[:], one[:])
        wcp_t = const.tile([128, 200 * H], _dt.bfloat16, name="wcpc")
        nc.sync.dma_start(wcp_t[:], wcp[:])
        flat = const.tile([128, gpc * 200], _dt.bfloat16, name="flatc")

        SCH = [(0, 128), (128, 128), (256, 128), (384, 17)]  # S chunks incl aug row

        for p in range(pairs):
            ga, gb = 2 * p, 2 * p + 1
            xa, xb, sa, sb = [], [], [], []
            for i, (o, k) in enumerate(KCH):
                t = inp.tile([k, NPG], _dt.bfloat16, name=f"xa{i}", tag=f"xa{i}")
                nc.sync.dma_start(t[:], xT[ga, o : o + k, :])
                xa.append(t)
                t = inp.tile([k, NPG], _dt.bfloat16, name=f"xb{i}", tag=f"xb{i}")
                nc.sync.dma_start(t[:], xT[gb, o : o + k, :])
                xb.append(t)
            for i, (o, k) in enumerate(SCH):
                t = inp.tile([k, NPG], _dt.bfloat16, name=f"sa{i}", tag=f"sa{i}")
                nc.sync.dma_start(t[:], Sa[ga, o : o + k, :])
                sa.append(t)
                t = inp.tile([k, NPG], _dt.bfloat16, name=f"sb{i}", tag=f"sb{i}")
                nc.sync.dma_start(t[:], Sa[gb, o : o + k, :])
                sb.append(t)
            drp = inp.tile([2, NPG], _dt.bfloat16, name="drp", tag="drp")
            nc.sync.dma_start(drp[:], dr[p])

            # ---- L1: tT = (x W1)^T, pair-packed in column groups ----
            ps = psz.tile([128, NPG], _dt.float32, name="psl1", tag="z")
            for i, (o, k) in enumerate(KCH):
                # interleave the two column groups so they stream concurrently
                nc.tensor.matmul(ps[0:64, :], w1_t[i][:], xa[i][:],
                                 start=(i == 0), stop=(i == 3), tile_position=(0, 0))
                nc.tensor.matmul(ps[64:128, :], w1_t[i][:], xb[i][:],
                                 start=(i == 0), stop=(i == 3), tile_position=(0, 64))

            for layer in (1, 2, 3):
                # evacuate tT (psum -> sbuf bf16); plain copy, D deferred
                tT = act.tile([128, NPG], _dt.bfloat16, name="tT", tag="tT")
                nc.scalar.activation(tT[:], ps[:], AF.Copy)

                # transpose chunks to node layout + evacuate with D scale
                pw = gpc if layer > 1 else 0  # dinv^2 cols for layers 2,3
                un = []
                for j, (o, k) in enumerate(KCH):
                    pt = pstr.tile([128, 128], _dt.bfloat16, name=f"pt{j}", tag=f"tr{j % 2}")
                    nc.tensor.transpose(pt[0:k, :], tT[:, o : o + k], idn_t[:])
                    rows = 17 if (j == 3 and layer < 3) else k
                    ut = un_p.tile([rows, 128], _dt.bfloat16, name=f"un{j}", tag=f"un{j}")
                    nc.vector.tensor_scalar(
                        ut[0:k, 0:64], pt[0:k, 0:64],
                        dck_t[j][0:k, pw + ga : pw + ga + 1], None, OP.mult)
                    nc.vector.tensor_scalar(
                        ut[0:k, 64:128], pt[0:k, 64:128],
                        dck_t[j][0:k, pw + gb : pw + gb + 1], None, OP.mult)
                    un.append(ut)
                if layer < 3:  # bias goes into the aug row of the tail chunk
                    nc.sync.dma_start(un[3][16:17, :], bia[layer - 1 : layer, :])

                # S-message matmul: z^T = u^T S (pair in column groups)
                ps2 = psz.tile([128, NPG], _dt.float32, name="psz2", tag="z")
                ntail = 17 if layer < 3 else 16
                for j in range(4):
                    k = KCH[j][1] if j < 3 else ntail
                    nc.tensor.matmul(ps2[0:64, :], un[j][0:k, 0:64], sa[j][0:k, :],
                                     start=(j == 0), stop=(j == 3), tile_position=(0, 0))
                    nc.tensor.matmul(ps2[64:128, :], un[j][0:k, 64:128], sb[j][0:k, :],
                                     start=(j == 0), stop=(j == 3), tile_position=(0, 64))

                if layer < 3:
                    aT = act.tile([128, NPG], _dt.bfloat16, name="aT", tag="aT")
                    nc.scalar.activation(aT[:], ps2[:], AF.Relu)
                    wt = w2_t if layer == 1 else w3_t
                    ps = psz.tile([128, NPG], _dt.float32, name="psw", tag="z")
                    nc.tensor.matmul(ps[0:64, :], wt[0:64, :], aT[0:64, :],
                                     start=True, stop=True, tile_position=(0, 0))
                    nc.tensor.matmul(ps[64:128, :], wt[64:128, :], aT[64:128, :],
                                     start=True, stop=True, tile_position=(64, 64))
                else:
                    # h3^T = z3^T * dinv (broadcast over feature partitions)
                    pd = pstr.tile([128, NPG], _dt.float32, name="pd", tag="pd", bufs=1)
                    nc.tensor.matmul(pd[:], c01_t[:], drp[:], start=True, stop=True)
                    dvb = act.tile([128, NPG], _dt.float32, name="dvb", tag="dvb")
                    nc.scalar.activation(dvb[:], pd[:], AF.Copy)
                    h3 = act.tile([128, NPG], _dt.bfloat16, name="h3", tag="h3")
                    nc.vector.tensor_tensor(h3[:], ps2[:], dvb[:], OP.mult)
                    # flatten: contiguous block copies into the readout tile
                    nc.sync.dma_start(flat[0:64, ga * 200 : ga * 200 + 200], h3[0:64, 0:200])
                    nc.sync.dma_start(flat[64:128, ga * 200 : ga * 200 + 200], h3[0:64, 200:400])
                    nc.sync.dma_start(flat[0:64, gb * 200 : gb * 200 + 200], h3[64:128, 0:200])
                    nc.sync.dma_start(flat[64:128, gb * 200 : gb * 200 + 200], h3[64:128, 200:400])

        # ---- readout: g = flat' . Wc' (contract 25600 in 200 chunks) ----
        flat_r = flat[:].rearrange("p (g c) -> p c g", g=gpc)
        gps = pstr.tile([gpc, H], _dt.float32, name="gps", tag="pd", bufs=1)
        for c in range(200):
            nc.tensor.matmul(gps[:], flat_r[:, c, :], wcp_t[:, c * H : (c + 1) * H],
                             start=(c == 0), stop=(c == 199))
        gsb = const.tile([gpc, H], _dt.bfloat16, name="gsb")
        nc.scalar.activation(gsb[:], gps[:], AF.Copy)
        gtp = pstr.tile([H, gpc], _dt.bfloat16, name="gtp", tag="tr0")
        nc.tensor.transpose(gtp[:], gsb[:], idn_t[0:gpc, 0:gpc])
        gts = const.tile([H, gpc], _dt.bfloat16, name="gts")
        nc.scalar.activation(gts[:], gtp[:], AF.Copy)
        ops = pstr.tile([gpc, 2], _dt.float32, name="ops", tag="tr1")
        nc.tensor.matmul(ops[:], gts[:], wl_t[:], start=True, stop=False)
        nc.tensor.matmul(ops[:], one_t[:], blr_t[:], start=False, stop=True)
        osb = const.tile([gpc, 2], _dt.float32, name="osb")
        nc.scalar.activation(osb[:], ops[:], AF.Copy)
        nc.sync.dma_start(out[:], osb[:])

    return nc


def build(gpc: int = GPC) -> bass.Bass:
    nc = bacc.Bacc("TRN2", target_bir_lowering=False, debug=False)
    _emit(nc, gpc)
    nc.compile()
    return nc


def prep_inputs(x, edge_index, edge_weight, W1, b1, W2, b2, W3, b3, Wc, bc, Wl, bl,
                gpc: int = GPC, ncores: int = NCORES):
    """Host-side prep: dense adjacency, normalization constants, layout."""
    f32 = np.float32
    x = np.asarray(x, f32)
    edge_index = np.asarray(edge_index)
    edge_weight = np.asarray(edge_weight, f32)
    W1, b1 = np.asarray(W1, f32), np.asarray(b1, f32)
    W2, b2 = np.asarray(W2, f32), np.asarray(b2, f32)
    W3, b3 = np.asarray(W3, f32), np.asarray(b3, f32)
    Wc, bc = np.asarray(Wc, f32), np.asarray(bc, f32)
    Wl, bl = np.asarray(Wl, f32), np.asarray(bl, f32)

    ng = gpc * ncores
    n = ng * NPG
    src, dst = edge_index[0], edge_index[1]
    S = np.zeros((n, NPG), f32)
    np.add.at(S, (src, dst - (src // NPG) * NPG), edge_weight)
    S[np.arange(n), np.arange(n) % NPG] += 1.0
    S3 = S.reshape(ng, NPG, NPG)
    deg = S3.sum(axis=1)
    dinv = (1.0 / np.sqrt(deg)).astype(f32)
    sqd = np.sqrt(deg).astype(f32)

    Sa = np.concatenate([S3, sqd[:, None, :]], axis=1).astype(BF)  # [ng,401,400]
    xT = np.ascontiguousarray(
        x.reshape(ng, NPG, FIN).transpose(0, 2, 1)).astype(BF)     # [ng,400,400]

    # dck [4, 128, 2*gpc] per core: dinv cols then dinv^2 cols
    pairs = gpc // 2
    dck_full = np.zeros((ncores, 4, 128, 2 * gpc), f32)
    dr_full = np.zeros((ncores, pairs, 2, NPG), f32)
    for c in range(ncores):
        dv = dinv[c * gpc : (c + 1) * gpc]          # [gpc, 400]
        for j, (o, k) in enumerate(KCH):
            dck_full[c, j, 0:k, 0:gpc] = dv[:, o : o + k].T
            dck_full[c, j, 0:k, gpc : 2 * gpc] = (dv * dv)[:, o : o + k].T
        dr_full[c] = dv.reshape(pairs, 2, NPG)

    # folded biases
    bc_p = bc + (np.tile(b3, NPG) @ Wc)
    bl_p = (bl + bc_p @ Wl).reshape(1, 2)

    # Wc' reorder to match device flat layout: chunk c rows 0:64 = node c,
    # rows 64:128 = node 200+c (features in order)
    Wcr = Wc.reshape(NPG, H, H)
    Wcp = np.zeros((200, 128, H), f32)
    Wcp[:, 0:64, :] = Wcr[0:200]
    Wcp[:, 64:128, :] = Wcr[200:400]
    wcp = np.ascontiguousarray(Wcp.transpose(1, 0, 2)).reshape(128, 200 * H).astype(BF)

    bia = np.zeros((2, 128), f32)
    bia[0] = np.concatenate([b1, b1])
    bia[1] = np.concatenate([b2, b2])

    c01 = np.zeros((2, 128), f32)
    c01[0, 0:64] = 1.0
    c01[1, 64:128] = 1.0

    consts = dict(
        w1=W1.astype(BF),
        w23=np.stack([W2, W3]).astype(BF),
        bia=bia.astype(BF),
        wcp=wcp,
        wl=Wl.astype(BF),
        blr=bl_p.astype(BF),
        idn=np.eye(128, dtype=f32).astype(BF),
        c01=c01.astype(BF),
        one=np.ones((1, gpc), f32).astype(BF),
    )

    in_maps = []
    for c in range(ncores):
        m = dict(consts)
        m["xT"] = xT[c * gpc : (c + 1) * gpc]
        m["Sa"] = Sa[c * gpc : (c + 1) * gpc]
        m["dck"] = dck_full[c]
        m["dr"] = dr_full[c].astype(BF)
        in_maps.append(m)
    return in_maps


_NC_CACHE = {}


def kernel(x, edge_index, edge_weight, W1, b1, W2, b2, W3, b3, Wc, bc, Wl, bl,
           _trace=False, _trace_kwargs=None):
    in_maps = prep_inputs(x, edge_index, edge_weight, W1, b1, W2, b2, W3, b3,
                          Wc, bc, Wl, bl)
    if GPC not in _NC_CACHE:
        _NC_CACHE[GPC] = build(GPC)
    nc = _NC_CACHE[GPC]
    res = run_bass_kernel_spmd(
        nc, in_maps, core_ids=list(range(NCORES)),
        trace=_trace, **(_trace_kwargs or {}))
    outs = np.concatenate([r["out"] for r in res.results], axis=0)
    if _trace:
        return outs.astype(np.float32), res
    return outs.astype(np.float32)
